# revision 34
# baseline (speedup 1.0000x reference)
"""Trainium2 Bass kernel for nn_CanadarmJacob (space-arm Jacobian, bm=1 path).

Contract: kernel(**inputs) takes FULL inputs (com_list (512,256,3,7) f32,
link_pose_list (512,256,4,4,9) f32, bm scalar) and returns the FULL output
(512,256,6,7) f32. Internally shards samples across 8 NeuronCores (pure data
parallel), runs a Bass/Tile kernel per core, and gathers.

v2 design: bf16 streams, act-major layout (P, comp, act, J) with J contiguous
so every big op hits the DVE 2x bf16 mode (0.52 ns/elem). Host packs only the
needed pose slices (rot gather + pos) -> 4.4x less input DMA. 3x3 smalls chain
stays f32. Activation engine carries the affine/copy side-channel (sign flip,
row duplication for shifted cross-product views, dtype casts, A-matrix spread).

Math (reformulated from the reference):
  rot   = pose[:3, AXIS[a], a], AXIS=[2,0,2,2,2,0,2]; rot[:,4] *= -1
  del   = com - pos ;  mdel = M del ; mcom = M com
  u     = {mdel_i del_j} (6) ; S = sum_a u ; scom = sum_a mcom (pairwise trees)
  w     = suffix-cumsum_a(mdel) ; jac = rot x del
  c     = scom/TM - BASE ; w2 = w - SM (x) c
  Hth   = D_suf . rot + w2 x jac ; jsm = SM jac
  H_s   = TM(cc^T - |c|^2 I) + diag(CD) + (trS) I - S   (3x3 symmetric)
  A     = -inv(H_s) via adjugate ; bot = A @ Hth ; top = -jsm/TM + c x bot
"""
import sys
import functools

if "/opt/trn_rl_repo" not in sys.path:
    sys.path.insert(0, "/opt/trn_rl_repo")

import numpy as np
import ml_dtypes

BF = ml_dtypes.bfloat16

# ---------------------------------------------------------------- constants
N_CORES = 8
P = 128          # SBUF partitions
J = 128          # samples per partition per core
A = 7            # actuated links
N_ACT = 7

AXIS = np.array([2, 0, 2, 2, 2, 0, 2])
MASS = np.array([105.98, 105.98, 314.98, 279.2, 105.98, 105.98, 243.66], np.float64)
TM = float(MASS.sum() + 100000.0 + 243.66)
DIAGS = np.array([[12.19, 12.19, 3.061], [12.19, 12.19, 3.061], [15.41, 2094.71, 2103.19],
                  [9.522, 1966.28, 1966.28], [8.305, 3.061, 8.0386], [12.13, 12.13, 3.061],
                  [9.336, 44.41, 44.41]], np.float64)
D_SUF = np.cumsum(DIAGS[::-1], axis=0)[::-1]          # (7,3) suffix inertia diag
SM = np.cumsum(MASS[::-1])[::-1]                      # (7,) suffix mass
CD = DIAGS.sum(axis=0)                                # (3,)
_TF0 = np.array([[1, 0, 0, 0], [0, -1, 0, 0], [0, 0, 1.3, 6], [0, 0, 0, 1]], np.float64)
_COM0 = np.array([[1, 0, 0, 0], [0, 1, 0, 0], [0, 0, 1, 0.5], [0, 0, 0, 1]], np.float64)
BASE = (_TF0 @ _COM0)[:3, 3] * 243.66 / (100000.0 + 243.66)   # [0, 0, ~0.0162]

# ctile rows (bf16, each (A, J) broadcast over J):
#   0=M, 1=SM, 2..4=D_suf[c], 5=-SM/TM
CT = np.broadcast_to(
    np.concatenate([MASS[None, :], SM[None, :], D_SUF.T,
                    (-SM / TM)[None, :]]).astype(np.float32)[:, :, None],
    (6, A, J)).astype(BF)
# f32 per-partition consts row: CD (3)
CONSTS32 = np.array(list(CD) + [float(BASE[2])], np.float32)
NC32 = CONSTS32.shape[0]


def _emit(nc, tc, ctx, dram):
    from concourse import mybir

    f32 = mybir.dt.float32
    b16 = mybir.dt.bfloat16
    OP = mybir.AluOpType
    V = nc.vector
    SE = nc.scalar           # Activation engine
    Copy = mybir.ActivationFunctionType.Copy

    pool = ctx.enter_context(tc.tile_pool(name="main", bufs=1))

    # ---- tiles (act-major: last dim J contiguous) -------------------------
    ctile = pool.tile([P, 6, A, J], b16)       # M, SM, Dx, Dy, Dz, -SM/TM
    c32 = pool.tile([P, NC32], f32)
    rot5 = pool.tile([P, 5, A, J], b16)        # rows 0-2 rot, 3-4 dup(x,y)
    pos = pool.tile([P, 3, A, J], b16)
    com = pool.tile([P, 3, A, J], b16)
    del5 = pool.tile([P, 5, A, J], b16)
    mdel = pool.tile([P, 3, A, J], b16)        # becomes w in place (suffix cumsum)
    prods = pool.tile([P, 9, A, J], b16)       # u rows 0-5 [xx,yy,zz,xy,yz,xz], mcom 6-8
    tl1 = pool.tile([P, 9, 3, J], b16)         # tree L1
    tc0 = pool.tile([P, 9, J], b16)            # tree L2 left
    tc1 = pool.tile([P, 9, J], b16)            # tree L2 right
    sums = pool.tile([P, 9, J], b16)           # S rows 0-5, scom rows 6-8
    jac5 = pool.tile([P, 5, A, J], b16)
    scr1 = pool.tile([P, 3, A, J], b16)
    scr2 = pool.tile([P, 3, A, J], b16)
    smc = pool.tile([P, 3, A, J], b16)
    w25 = pool.tile([P, 5, A, J], b16)
    hth = pool.tile([P, 3, A, J], b16)
    m9 = pool.tile([P, 9, A, J], b16)
    outt = pool.tile([P, 2, 8, A, J // 2], b16)   # rows: top 0-2, bot 3-5, dup 6-7
    c5 = pool.tile([P, 5, J], f32)             # c rows x,y,z,x,y (f32)
    cb5 = pool.tile([P, 5, J], b16)            # c in bf16 + dup
    sm = pool.tile([P, 26, J], f32)            # smalls scratch
    a9 = pool.tile([P, 9, J], b16)             # A row-major 3x3

    # smalls row map (sm tile)
    CC = 0      # rows 0-2 diag(xx,yy,zz), 3-5 off (xy,yz,xz)
    CSQ = 6
    SSR = 7
    T3 = 8      # rows 8-10
    D3 = 11     # rows 11-13
    HS = 14     # rows 14-19: [h00,h11,h22,h01,h12,h02]
    ADJ = 20    # rows 20-25 order [A00,A01,A02,A11,A12,A22]
    MA = 8      # scratch pair rows 8-9 (T3 dead after HS built)
    MB = 10     # scratch pair rows 10-11
    DET = 12
    RDET = 13

    # ---- input DMAs (pos/com first so compute starts early) --------------
    JH = J // 2
    nc.sync.dma_start(out=pos[:], in_=dram["pos"][:])
    nc.scalar.dma_start(out=com[:], in_=dram["com"][:])
    nc.scalar.dma_start(out=ctile[:, 0:1], in_=dram["ctm"][:])
    nc.sync.dma_start(out=rot5[:, 0:3], in_=dram["rot"][:])
    nc.sync.dma_start(out=ctile[:, 1:6], in_=dram["ctr"][:])
    nc.sync.dma_start(out=c32[:], in_=dram["c32"][:])

    SE.copy(rot5[:, 3:5], rot5[:, 0:2])       # dup unflipped rows first

    Mb = ctile[:, 0].unsqueeze(1).broadcast_to((P, 3, A, J))
    SMb = ctile[:, 1].unsqueeze(1).broadcast_to((P, 3, A, J))
    Dt = ctile[:, 2:5]

    # ---- streams (DVE unless noted) --------------------------------------
    V.tensor_tensor(out=del5[:, 0:3], in0=com[:], in1=pos[:], op=OP.subtract)
    SE.copy(del5[:, 3:5], del5[:, 0:2])
    V.tensor_tensor(out=mdel[:], in0=Mb, in1=del5[:, 0:3], op=OP.mult)
    V.tensor_tensor(out=prods[:, 6:9], in0=Mb, in1=com[:], op=OP.mult)
    V.tensor_tensor(out=prods[:, 0:3], in0=mdel[:], in1=del5[:, 0:3], op=OP.mult)
    V.tensor_tensor(out=prods[:, 3:5], in0=mdel[:, 0:2], in1=del5[:, 1:3], op=OP.mult)
    V.tensor_tensor(out=prods[:, 5], in0=mdel[:, 0], in1=del5[:, 2], op=OP.mult)

    # tree L1 first (no rot dependency): covers the rot DMA arrival
    V.tensor_tensor(out=tl1[:], in0=prods[:, :, 0:3], in1=prods[:, :, 4:7], op=OP.add)

    # sign-flip act 4 of rot (all 5 rows incl dups) on DVE, then jac cross
    V.tensor_scalar_mul(rot5[:, :, 4], rot5[:, :, 4], -1.0)
    V.tensor_tensor(out=scr1[:], in0=rot5[:, 1:4], in1=del5[:, 2:5], op=OP.mult)
    V.tensor_tensor(out=scr2[:], in0=rot5[:, 2:5], in1=del5[:, 1:4], op=OP.mult)
    V.tensor_tensor(out=jac5[:, 0:3], in0=scr1[:], in1=scr2[:], op=OP.subtract)
    SE.copy(jac5[:, 3:5], jac5[:, 0:2])

    # rest of the act-sum tree -> S rows 0-5, scom rows 6-8
    V.tensor_tensor(out=tc0[:], in0=tl1[:, :, 0], in1=tl1[:, :, 1], op=OP.add)
    V.tensor_tensor(out=tc1[:], in0=tl1[:, :, 2], in1=prods[:, :, 3], op=OP.add)
    V.tensor_tensor(out=sums[:], in0=tc0[:], in1=tc1[:], op=OP.add)

    # w: suffix cumsum over acts, in place in mdel
    for k in range(A - 2, -1, -1):
        V.tensor_tensor(out=mdel[:, :, k], in0=mdel[:, :, k], in1=mdel[:, :, k + 1],
                        op=OP.add)

    # c = scom/TM - BASE (Act engine), then bf16 copy + dups
    SE.mul(c5[:, 0:2], sums[:, 6:8], 1.0 / TM)
    SE.activation(c5[:, 2], sums[:, 8], Copy, bias=-float(BASE[2]), scale=1.0 / TM)
    SE.copy(c5[:, 3:5], c5[:, 0:2])
    SE.copy(cb5[:, 0:3], c5[:, 0:3])
    SE.copy(cb5[:, 3:5], cb5[:, 0:2])

    # w2 = w - SM (x) c
    cbb = cb5[:, 0:3].unsqueeze(2).broadcast_to((P, 3, A, J))
    V.tensor_tensor(out=smc[:], in0=SMb, in1=cbb, op=OP.mult)
    V.tensor_tensor(out=w25[:, 0:3], in0=mdel[:], in1=smc[:], op=OP.subtract)
    SE.copy(w25[:, 3:5], w25[:, 0:2])

    # Hth = D.rot + w2 x jac
    V.tensor_tensor(out=smc[:], in0=Dt, in1=rot5[:, 0:3], op=OP.mult)
    V.tensor_tensor(out=scr1[:], in0=w25[:, 1:4], in1=jac5[:, 2:5], op=OP.mult)
    V.tensor_tensor(out=scr2[:], in0=w25[:, 2:5], in1=jac5[:, 1:4], op=OP.mult)
    V.tensor_tensor(out=scr1[:], in0=scr1[:], in1=scr2[:], op=OP.subtract)
    V.tensor_tensor(out=hth[:], in0=smc[:], in1=scr1[:], op=OP.add)

    # ---- smalls (f32) ----------------------------------------------------
    # cc products via dup'd c5: diag then off [xy,yz,xz]
    V.tensor_tensor(out=sm[:, CC:CC + 3], in0=c5[:, 0:3], in1=c5[:, 0:3], op=OP.mult)
    V.tensor_tensor(out=sm[:, CC + 3:CC + 6], in0=c5[:, 0:3], in1=c5[:, 1:4], op=OP.mult)
    V.tensor_tensor(out=sm[:, CSQ], in0=sm[:, CC], in1=sm[:, CC + 1], op=OP.add)
    V.tensor_tensor(out=sm[:, CSQ], in0=sm[:, CSQ], in1=sm[:, CC + 2], op=OP.add)
    V.tensor_tensor(out=sm[:, SSR], in0=sums[:, 0], in1=sums[:, 1], op=OP.add)
    V.tensor_tensor(out=sm[:, SSR], in0=sm[:, SSR], in1=sums[:, 2], op=OP.add)

    CDb = c32[:, 0:3].unsqueeze(2).broadcast_to((P, 3, J))
    ssb = sm[:, SSR].unsqueeze(1).broadcast_to((P, 3, J))
    csqb = sm[:, CSQ].unsqueeze(1).broadcast_to((P, 3, J))
    V.tensor_tensor(out=sm[:, T3:T3 + 3], in0=CDb, in1=sums[:, 0:3], op=OP.subtract)
    V.tensor_tensor(out=sm[:, T3:T3 + 3], in0=sm[:, T3:T3 + 3], in1=ssb, op=OP.add)
    V.tensor_tensor(out=sm[:, D3:D3 + 3], in0=sm[:, CC:CC + 3], in1=csqb, op=OP.subtract)
    V.scalar_tensor_tensor(out=sm[:, HS:HS + 3], in0=sm[:, D3:D3 + 3], scalar=TM,
                           in1=sm[:, T3:T3 + 3], op0=OP.mult, op1=OP.add)
    V.scalar_tensor_tensor(out=sm[:, HS + 3:HS + 6], in0=sm[:, CC + 3:CC + 6],
                           scalar=TM, in1=sums[:, 3:6], op0=OP.mult, op1=OP.subtract)

    # adjugate of Hs rows [h00,h11,h22,h01,h12,h02] -> adj [A00,A01,A02,A11,A12,A22]
    h = lambda i: sm[:, HS + i]
    m2a = sm[:, MA:MA + 2]
    m2b = sm[:, MB:MB + 2]
    # A00 = h11 h22 - h12^2 ; A11 = h00 h22 - h02^2
    V.tensor_tensor(out=m2a, in0=sm[:, HS + 1:HS - 1:-1],
                    in1=h(2).unsqueeze(1).broadcast_to((P, 2, J)), op=OP.mult)
    V.tensor_tensor(out=m2b, in0=sm[:, HS + 4:HS + 6],
                    in1=sm[:, HS + 4:HS + 6], op=OP.mult)
    V.tensor_tensor(out=sm[:, ADJ:ADJ + 4:3], in0=m2a, in1=m2b, op=OP.subtract)
    # A22 = h00 h11 - h01^2 ; A02 = h01 h12 - h02 h11
    V.tensor_tensor(out=m2a, in0=sm[:, HS:HS + 4:3], in1=sm[:, HS + 1:HS + 5:3],
                    op=OP.mult)
    V.tensor_tensor(out=m2b, in0=sm[:, HS + 3:HS + 6:2], in1=sm[:, HS + 3:HS - 1:-2],
                    op=OP.mult)
    V.tensor_tensor(out=sm[:, ADJ + 5:ADJ + 1:-3], in0=m2a, in1=m2b, op=OP.subtract)
    # [A01, A12] batched: firsts [h02*h12, h01*h02], seconds [h01*h22, h12*h00]
    V.tensor_tensor(out=m2a, in0=sm[:, HS + 5:HS + 2:-2], in1=sm[:, HS + 4:HS + 6],
                    op=OP.mult)
    V.tensor_tensor(out=m2b, in0=sm[:, HS + 3:HS + 5], in1=sm[:, HS + 2:HS - 1:-2],
                    op=OP.mult)
    V.tensor_tensor(out=sm[:, ADJ + 1:ADJ + 5:3], in0=m2a, in1=m2b, op=OP.subtract)

    # det = h00 A00 + h01 A01 + h02 A02 ; A(bf16) = adj * (-1/det)
    V.tensor_tensor(out=sm[:, DET], in0=h(0), in1=sm[:, ADJ], op=OP.mult)
    V.tensor_tensor(out=sm[:, MA], in0=h(3), in1=sm[:, ADJ + 1], op=OP.mult)
    V.tensor_tensor(out=sm[:, DET], in0=sm[:, DET], in1=sm[:, MA], op=OP.add)
    V.tensor_tensor(out=sm[:, MB], in0=h(5), in1=sm[:, ADJ + 2], op=OP.mult)
    V.tensor_tensor(out=sm[:, DET], in0=sm[:, DET], in1=sm[:, MB], op=OP.add)
    V.reciprocal(out=sm[:, RDET], in_=sm[:, DET])
    # A = adj * (-1/det) as bf16, written straight into row-major 3x3 rows
    # [00,01,02,10,11,12,20,21,22]: upper entries by STT, mirrors by copy
    V.scalar_tensor_tensor(out=a9[:, 0:3], in0=sm[:, ADJ:ADJ + 3], scalar=-1.0,
                           in1=sm[:, RDET].unsqueeze(1).broadcast_to((P, 3, J)),
                           op0=OP.mult, op1=OP.mult)
    V.scalar_tensor_tensor(out=a9[:, 4:6], in0=sm[:, ADJ + 3:ADJ + 5], scalar=-1.0,
                           in1=sm[:, RDET].unsqueeze(1).broadcast_to((P, 2, J)),
                           op0=OP.mult, op1=OP.mult)
    V.scalar_tensor_tensor(out=a9[:, 8], in0=sm[:, ADJ + 5], scalar=-1.0,
                           in1=sm[:, RDET], op0=OP.mult, op1=OP.mult)
    V.tensor_copy(out=a9[:, 3], in_=a9[:, 1])
    V.tensor_copy(out=a9[:, 6], in_=a9[:, 2])
    V.tensor_copy(out=a9[:, 7], in_=a9[:, 5])

    # ---- bot = A @ Hth (full J: keeps the big product op 3D-mergeable) ----
    m9v = m9[:].rearrange("p (r c) a j -> p r c a j", r=3)
    a9v = a9[:].rearrange("p (r c) j -> p r c j", r=3).unsqueeze(3) \
        .broadcast_to((P, 3, 3, A, J))
    hthv = hth[:].unsqueeze(1).broadcast_to((P, 3, 3, A, J))
    V.tensor_tensor(out=m9v, in0=a9v, in1=hthv, op=OP.mult)

    # ---- bot sums, top = -(SM/TM) jac + c x bot, per J-half --------------
    for hj, hjs in ((0, slice(0, JH)), (1, slice(JH, J))):
        SMnb = ctile[:, 5, :, hjs].unsqueeze(1).broadcast_to((P, 3, A, JH))
        V.tensor_tensor(out=outt[:, hj, 3:6], in0=m9[:, 0:9:3, :, hjs],
                        in1=m9[:, 1:9:3, :, hjs], op=OP.add)
        V.tensor_tensor(out=outt[:, hj, 3:6], in0=outt[:, hj, 3:6],
                        in1=m9[:, 2:9:3, :, hjs], op=OP.add)
        V.tensor_copy(out=outt[:, hj, 6:8], in_=outt[:, hj, 3:5])
        cbb1 = cb5[:, 1:4, hjs].unsqueeze(2).broadcast_to((P, 3, A, JH))
        cbb2 = cb5[:, 2:5, hjs].unsqueeze(2).broadcast_to((P, 3, A, JH))
        V.tensor_tensor(out=scr1[:, :, :, hjs], in0=cbb1,
                        in1=outt[:, hj, 5:8], op=OP.mult)
        V.tensor_tensor(out=scr2[:, :, :, hjs], in0=cbb2,
                        in1=outt[:, hj, 4:7], op=OP.mult)
        V.tensor_tensor(out=scr1[:, :, :, hjs], in0=scr1[:, :, :, hjs],
                        in1=scr2[:, :, :, hjs], op=OP.subtract)
        V.tensor_tensor(out=scr2[:, :, :, hjs], in0=SMnb,
                        in1=jac5[:, 0:3, :, hjs], op=OP.mult)
        V.tensor_tensor(out=outt[:, hj, 0:3], in0=scr1[:, :, :, hjs],
                        in1=scr2[:, :, :, hjs], op=OP.add)
        nc.sync.dma_start(out=dram["out"][:, hj], in_=outt[:, hj, 0:6])


@functools.lru_cache(maxsize=1)
def _program():
    from contextlib import ExitStack
    import concourse.bacc as bacc
    import concourse.tile as tile
    from concourse import mybir

    f32 = mybir.dt.float32
    b16 = mybir.dt.bfloat16
    nc = bacc.Bacc("TRN2", target_bir_lowering=False, debug=False)
    JH = J // 2
    dram = {
        "rot": nc.dram_tensor("rot", [P, 3, A, J], b16, kind="ExternalInput"),
        "pos": nc.dram_tensor("pos", [P, 3, A, J], b16, kind="ExternalInput"),
        "com": nc.dram_tensor("com", [P, 3, A, J], b16, kind="ExternalInput"),
        "ctm": nc.dram_tensor("ctm", [P, 1, A, J], b16, kind="ExternalInput"),
        "ctr": nc.dram_tensor("ctr", [P, 5, A, J], b16, kind="ExternalInput"),
        "c32": nc.dram_tensor("c32", [P, NC32], f32, kind="ExternalInput"),
        "out": nc.dram_tensor("out", [P, 2, 6, A, JH], b16, kind="ExternalOutput"),
    }
    with tile.TileContext(nc) as tc:
        with ExitStack() as ctx:
            _emit(nc, tc, ctx, dram)
    nc.compile()
    return nc


def pack_inputs(com_list, link_pose_list):
    """Host-side layout packing (pure data movement + dtype cast)."""
    N = N_CORES * P * J
    pose = np.ascontiguousarray(link_pose_list, dtype=np.float32).reshape(N, 4, 4, 9)
    com = np.ascontiguousarray(com_list, dtype=np.float32).reshape(N, 3, 7)
    rot = pose[:, :3, AXIS, np.arange(7)]                # (N, 3, 7)
    pos = pose[:, :3, 3, :7]                             # (N, 3, 7)

    def to_core_layout(x):  # (N, 3, 7) -> (cores, P, 3, 7, J)
        return np.ascontiguousarray(
            x.reshape(N_CORES, P, J, 3, 7).transpose(0, 1, 3, 4, 2)).astype(BF)

    return to_core_layout(rot), to_core_layout(pos), to_core_layout(com)


def make_in_maps(rot, pos, com):
    ctm = np.ascontiguousarray(np.broadcast_to(CT[0:1], (P, 1, A, J)))
    ctr = np.ascontiguousarray(np.broadcast_to(CT[1:6], (P, 5, A, J)))
    c32 = np.broadcast_to(CONSTS32, (P, NC32)).copy()
    return [
        {"rot": rot[k], "pos": pos[k], "com": com[k], "ctm": ctm, "ctr": ctr,
         "c32": c32}
        for k in range(N_CORES)
    ]


def unpack_output(res):
    out = np.stack([res.results[k]["out"] for k in range(N_CORES)])  # (8,P,2,6,7,JH)
    out = out.astype(np.float32).transpose(0, 1, 2, 5, 3, 4)         # (8,P,2,JH,6,7)
    return np.ascontiguousarray(out.reshape(512, 256, 6, 7))


def _kernel_bm0(com, pose):
    # bm=0 path (not exercised by the shipped setup_inputs; numpy fallback)
    rot = pose[:, :, :3, 2, :N_ACT].copy()
    rot[..., 1] = pose[:, :, :3, 0, 1]
    rot[..., 5] = pose[:, :, :3, 0, 5]
    rot[..., 4] *= -1.0
    delp = pose[:, :, :3, 3, -2][..., None] - pose[:, :, :3, 3, :N_ACT]
    jt = np.cross(rot, delp, axis=2)
    return np.concatenate([jt, rot], axis=2).astype(np.float32)


def kernel(com_list, link_pose_list, bm):
    if not int(bm):
        return _kernel_bm0(np.asarray(com_list, np.float32),
                           np.asarray(link_pose_list, np.float32))

    from concourse.bass_utils import run_bass_kernel_spmd

    nc = _program()
    rot, pos, com = pack_inputs(com_list, link_pose_list)
    res = run_bass_kernel_spmd(nc, make_in_maps(rot, pos, com),
                               core_ids=list(range(N_CORES)))
    return unpack_output(res)


# revision 35
# speedup vs baseline: 1.0272x; 1.0272x over previous
"""Trainium2 Bass kernel for nn_CanadarmJacob (space-arm Jacobian, bm=1 path).

Contract: kernel(**inputs) takes FULL inputs (com_list (512,256,3,7) f32,
link_pose_list (512,256,4,4,9) f32, bm scalar) and returns the FULL output
(512,256,6,7) f32. Internally shards samples across 8 NeuronCores (pure data
parallel), runs a Bass/Tile kernel per core, and gathers.

v2 design: bf16 streams, act-major layout (P, comp, act, J) with J contiguous
so every big op hits the DVE 2x bf16 mode (0.52 ns/elem). Host packs only the
needed pose slices (rot gather + pos) -> 4.4x less input DMA. 3x3 smalls chain
stays f32. Activation engine carries the affine/copy side-channel (sign flip,
row duplication for shifted cross-product views, dtype casts, A-matrix spread).

Math (reformulated from the reference):
  rot   = pose[:3, AXIS[a], a], AXIS=[2,0,2,2,2,0,2]; rot[:,4] *= -1
  del   = com - pos ;  mdel = M del ; mcom = M com
  u     = {mdel_i del_j} (6) ; S = sum_a u ; scom = sum_a mcom (pairwise trees)
  w     = suffix-cumsum_a(mdel) ; jac = rot x del
  c     = scom/TM - BASE ; w2 = w - SM (x) c
  Hth   = D_suf . rot + w2 x jac ; jsm = SM jac
  H_s   = TM(cc^T - |c|^2 I) + diag(CD) + (trS) I - S   (3x3 symmetric)
  A     = -inv(H_s) via adjugate ; bot = A @ Hth ; top = -jsm/TM + c x bot
"""
import sys
import functools

if "/opt/trn_rl_repo" not in sys.path:
    sys.path.insert(0, "/opt/trn_rl_repo")

import numpy as np
import ml_dtypes

BF = ml_dtypes.bfloat16

# ---------------------------------------------------------------- constants
N_CORES = 8
P = 128          # SBUF partitions
J = 128          # samples per partition per core
A = 7            # actuated links
N_ACT = 7

AXIS = np.array([2, 0, 2, 2, 2, 0, 2])
MASS = np.array([105.98, 105.98, 314.98, 279.2, 105.98, 105.98, 243.66], np.float64)
TM = float(MASS.sum() + 100000.0 + 243.66)
DIAGS = np.array([[12.19, 12.19, 3.061], [12.19, 12.19, 3.061], [15.41, 2094.71, 2103.19],
                  [9.522, 1966.28, 1966.28], [8.305, 3.061, 8.0386], [12.13, 12.13, 3.061],
                  [9.336, 44.41, 44.41]], np.float64)
D_SUF = np.cumsum(DIAGS[::-1], axis=0)[::-1]          # (7,3) suffix inertia diag
SM = np.cumsum(MASS[::-1])[::-1]                      # (7,) suffix mass
CD = DIAGS.sum(axis=0)                                # (3,)
_TF0 = np.array([[1, 0, 0, 0], [0, -1, 0, 0], [0, 0, 1.3, 6], [0, 0, 0, 1]], np.float64)
_COM0 = np.array([[1, 0, 0, 0], [0, 1, 0, 0], [0, 0, 1, 0.5], [0, 0, 0, 1]], np.float64)
BASE = (_TF0 @ _COM0)[:3, 3] * 243.66 / (100000.0 + 243.66)   # [0, 0, ~0.0162]

# ctile rows (bf16, each (A, J) broadcast over J):
#   0=M, 1=SM, 2..4=D_suf[c], 5=-SM/TM
CT = np.broadcast_to(
    np.concatenate([MASS[None, :], SM[None, :], D_SUF.T,
                    (-SM / TM)[None, :]]).astype(np.float32)[:, :, None],
    (6, A, J)).astype(BF)
# f32 per-partition consts row: CD (3)
CONSTS32 = np.array(list(CD) + [float(BASE[2])], np.float32)
NC32 = CONSTS32.shape[0]


def _emit(nc, tc, ctx, dram):
    from concourse import mybir

    f32 = mybir.dt.float32
    b16 = mybir.dt.bfloat16
    OP = mybir.AluOpType
    V = nc.vector
    SE = nc.scalar           # Activation engine
    Copy = mybir.ActivationFunctionType.Copy

    pool = ctx.enter_context(tc.tile_pool(name="main", bufs=1))

    # ---- tiles (act-major: last dim J contiguous) -------------------------
    ctile = pool.tile([P, 6, A, J], b16)       # M, SM, Dx, Dy, Dz, -SM/TM
    c32 = pool.tile([P, NC32], f32)
    rot5 = pool.tile([P, 5, A, J], b16)        # rows 0-2 rot, 3-4 dup(x,y)
    pos = pool.tile([P, 3, A, J], b16)
    com = pool.tile([P, 3, A, J], b16)
    del5 = pool.tile([P, 5, A, J], b16)
    mdel = pool.tile([P, 3, A, J], b16)        # becomes w in place (suffix cumsum)
    prods = pool.tile([P, 9, A, J], b16)       # u rows 0-5 [xx,yy,zz,xy,yz,xz], mcom 6-8
    tl1 = pool.tile([P, 9, 3, J], b16)         # tree L1
    tc0 = pool.tile([P, 9, J], b16)            # tree L2 left
    tc1 = pool.tile([P, 9, J], b16)            # tree L2 right
    sums = pool.tile([P, 9, J], b16)           # S rows 0-5, scom rows 6-8
    jac5 = pool.tile([P, 5, A, J], b16)
    scr1 = pool.tile([P, 3, A, J], b16)
    scr2 = pool.tile([P, 3, A, J], b16)
    smc = pool.tile([P, 3, A, J], b16)
    w25 = pool.tile([P, 5, A, J], b16)
    hth = pool.tile([P, 3, A, J], b16)
    m9 = pool.tile([P, 9, A, J], b16)
    outt = pool.tile([P, 2, 8, A, J // 2], b16)   # rows: top 0-2, bot 3-5, dup 6-7
    c5 = pool.tile([P, 5, J], f32)             # c rows x,y,z,x,y (f32)
    cb5 = pool.tile([P, 5, J], b16)            # c in bf16 + dup
    sm = pool.tile([P, 26, J], f32)            # smalls scratch
    a9 = pool.tile([P, 9, J], b16)             # A row-major 3x3

    # smalls row map (sm tile)
    CC = 0      # rows 0-2 diag(xx,yy,zz), 3-5 off (xy,yz,xz)
    CSQ = 6
    SSR = 7
    T3 = 8      # rows 8-10
    D3 = 11     # rows 11-13
    HS = 14     # rows 14-19: [h00,h11,h22,h01,h12,h02]
    ADJ = 20    # rows 20-25 order [A00,A01,A02,A11,A12,A22]
    MA = 8      # scratch pair rows 8-9 (T3 dead after HS built)
    MB = 10     # scratch pair rows 10-11
    DET = 12
    RDET = 13

    # ---- input DMAs (pos/com first so compute starts early) --------------
    JH = J // 2
    nc.sync.dma_start(out=pos[:], in_=dram["pos"][:])
    nc.scalar.dma_start(out=com[:], in_=dram["com"][:])
    nc.scalar.dma_start(out=ctile[:, 0:1], in_=dram["ctm"][:])
    nc.sync.dma_start(out=rot5[:, 0:3], in_=dram["rot"][:])
    nc.sync.dma_start(out=ctile[:, 1:6], in_=dram["ctr"][:])
    nc.sync.dma_start(out=c32[:], in_=dram["c32"][:])

    SE.copy(rot5[:, 3:5], rot5[:, 0:2])       # dup unflipped rows first

    Mb = ctile[:, 0].unsqueeze(1).broadcast_to((P, 3, A, J))
    SMb = ctile[:, 1].unsqueeze(1).broadcast_to((P, 3, A, J))
    Dt = ctile[:, 2:5]

    # ---- streams (DVE unless noted) --------------------------------------
    V.tensor_tensor(out=del5[:, 0:3], in0=com[:], in1=pos[:], op=OP.subtract)
    SE.copy(del5[:, 3:5], del5[:, 0:2])
    V.tensor_tensor(out=mdel[:], in0=Mb, in1=del5[:, 0:3], op=OP.mult)
    V.tensor_tensor(out=prods[:, 6:9], in0=Mb, in1=com[:], op=OP.mult)
    V.tensor_tensor(out=prods[:, 0:3], in0=mdel[:], in1=del5[:, 0:3], op=OP.mult)
    V.tensor_tensor(out=prods[:, 3:5], in0=mdel[:, 0:2], in1=del5[:, 1:3], op=OP.mult)
    V.tensor_tensor(out=prods[:, 5], in0=mdel[:, 0], in1=del5[:, 2], op=OP.mult)

    # sign-flip act 4 of rot (all 5 rows incl dups) on DVE, then jac cross
    V.tensor_scalar_mul(rot5[:, :, 4], rot5[:, :, 4], -1.0)
    V.tensor_tensor(out=scr1[:], in0=rot5[:, 1:4], in1=del5[:, 2:5], op=OP.mult)
    V.tensor_tensor(out=scr2[:], in0=rot5[:, 2:5], in1=del5[:, 1:4], op=OP.mult)
    V.tensor_tensor(out=jac5[:, 0:3], in0=scr1[:], in1=scr2[:], op=OP.subtract)
    SE.copy(jac5[:, 3:5], jac5[:, 0:2])

    # pairwise act-sum tree over prods: (7) -> S rows 0-5, scom rows 6-8
    V.tensor_tensor(out=tl1[:], in0=prods[:, :, 0:3], in1=prods[:, :, 4:7], op=OP.add)
    V.tensor_tensor(out=tc0[:], in0=tl1[:, :, 0], in1=tl1[:, :, 1], op=OP.add)
    V.tensor_tensor(out=tc1[:], in0=tl1[:, :, 2], in1=prods[:, :, 3], op=OP.add)
    V.tensor_tensor(out=sums[:], in0=tc0[:], in1=tc1[:], op=OP.add)

    # w: suffix cumsum over acts, in place in mdel
    for k in range(A - 2, -1, -1):
        V.tensor_tensor(out=mdel[:, :, k], in0=mdel[:, :, k], in1=mdel[:, :, k + 1],
                        op=OP.add)

    # c = scom/TM - BASE (Act engine), then bf16 copy + dups
    SE.mul(c5[:, 0:2], sums[:, 6:8], 1.0 / TM)
    SE.activation(c5[:, 2], sums[:, 8], Copy, bias=-float(BASE[2]), scale=1.0 / TM)
    SE.copy(c5[:, 3:5], c5[:, 0:2])
    SE.copy(cb5[:, 0:3], c5[:, 0:3])
    SE.copy(cb5[:, 3:5], cb5[:, 0:2])

    # w2 = w - SM (x) c
    cbb = cb5[:, 0:3].unsqueeze(2).broadcast_to((P, 3, A, J))
    V.tensor_tensor(out=smc[:], in0=SMb, in1=cbb, op=OP.mult)
    V.tensor_tensor(out=w25[:, 0:3], in0=mdel[:], in1=smc[:], op=OP.subtract)
    SE.copy(w25[:, 3:5], w25[:, 0:2])

    # Hth = D.rot + w2 x jac
    V.tensor_tensor(out=smc[:], in0=Dt, in1=rot5[:, 0:3], op=OP.mult)
    V.tensor_tensor(out=scr1[:], in0=w25[:, 1:4], in1=jac5[:, 2:5], op=OP.mult)
    V.tensor_tensor(out=scr2[:], in0=w25[:, 2:5], in1=jac5[:, 1:4], op=OP.mult)
    V.tensor_tensor(out=scr1[:], in0=scr1[:], in1=scr2[:], op=OP.subtract)
    V.tensor_tensor(out=hth[:], in0=smc[:], in1=scr1[:], op=OP.add)

    # ---- smalls (f32) ----------------------------------------------------
    # cc products via dup'd c5: diag then off [xy,yz,xz]
    V.tensor_tensor(out=sm[:, CC:CC + 3], in0=c5[:, 0:3], in1=c5[:, 0:3], op=OP.mult)
    V.tensor_tensor(out=sm[:, CC + 3:CC + 6], in0=c5[:, 0:3], in1=c5[:, 1:4], op=OP.mult)
    V.tensor_tensor(out=sm[:, CSQ], in0=sm[:, CC], in1=sm[:, CC + 1], op=OP.add)
    V.tensor_tensor(out=sm[:, CSQ], in0=sm[:, CSQ], in1=sm[:, CC + 2], op=OP.add)
    V.tensor_tensor(out=sm[:, SSR], in0=sums[:, 0], in1=sums[:, 1], op=OP.add)
    V.tensor_tensor(out=sm[:, SSR], in0=sm[:, SSR], in1=sums[:, 2], op=OP.add)

    CDb = c32[:, 0:3].unsqueeze(2).broadcast_to((P, 3, J))
    ssb = sm[:, SSR].unsqueeze(1).broadcast_to((P, 3, J))
    csqb = sm[:, CSQ].unsqueeze(1).broadcast_to((P, 3, J))
    V.tensor_tensor(out=sm[:, T3:T3 + 3], in0=CDb, in1=sums[:, 0:3], op=OP.subtract)
    V.tensor_tensor(out=sm[:, T3:T3 + 3], in0=sm[:, T3:T3 + 3], in1=ssb, op=OP.add)
    V.tensor_tensor(out=sm[:, D3:D3 + 3], in0=sm[:, CC:CC + 3], in1=csqb, op=OP.subtract)
    V.scalar_tensor_tensor(out=sm[:, HS:HS + 3], in0=sm[:, D3:D3 + 3], scalar=TM,
                           in1=sm[:, T3:T3 + 3], op0=OP.mult, op1=OP.add)
    V.scalar_tensor_tensor(out=sm[:, HS + 3:HS + 6], in0=sm[:, CC + 3:CC + 6],
                           scalar=TM, in1=sums[:, 3:6], op0=OP.mult, op1=OP.subtract)

    # adjugate of Hs rows [h00,h11,h22,h01,h12,h02] -> adj [A00,A01,A02,A11,A12,A22]
    h = lambda i: sm[:, HS + i]
    m2a = sm[:, MA:MA + 2]
    m2b = sm[:, MB:MB + 2]
    # A00 = h11 h22 - h12^2 ; A11 = h00 h22 - h02^2
    V.tensor_tensor(out=m2a, in0=sm[:, HS + 1:HS - 1:-1],
                    in1=h(2).unsqueeze(1).broadcast_to((P, 2, J)), op=OP.mult)
    V.tensor_tensor(out=m2b, in0=sm[:, HS + 4:HS + 6],
                    in1=sm[:, HS + 4:HS + 6], op=OP.mult)
    V.tensor_tensor(out=sm[:, ADJ:ADJ + 4:3], in0=m2a, in1=m2b, op=OP.subtract)
    # A22 = h00 h11 - h01^2 ; A02 = h01 h12 - h02 h11
    V.tensor_tensor(out=m2a, in0=sm[:, HS:HS + 4:3], in1=sm[:, HS + 1:HS + 5:3],
                    op=OP.mult)
    V.tensor_tensor(out=m2b, in0=sm[:, HS + 3:HS + 6:2], in1=sm[:, HS + 3:HS - 1:-2],
                    op=OP.mult)
    V.tensor_tensor(out=sm[:, ADJ + 5:ADJ + 1:-3], in0=m2a, in1=m2b, op=OP.subtract)
    # [A01, A12] batched: firsts [h02*h12, h01*h02], seconds [h01*h22, h12*h00]
    V.tensor_tensor(out=m2a, in0=sm[:, HS + 5:HS + 2:-2], in1=sm[:, HS + 4:HS + 6],
                    op=OP.mult)
    V.tensor_tensor(out=m2b, in0=sm[:, HS + 3:HS + 5], in1=sm[:, HS + 2:HS - 1:-2],
                    op=OP.mult)
    V.tensor_tensor(out=sm[:, ADJ + 1:ADJ + 5:3], in0=m2a, in1=m2b, op=OP.subtract)

    # det = h00 A00 + h01 A01 + h02 A02 ; A(bf16) = adj * (-1/det)
    V.tensor_tensor(out=sm[:, DET], in0=h(0), in1=sm[:, ADJ], op=OP.mult)
    V.tensor_tensor(out=sm[:, MA], in0=h(3), in1=sm[:, ADJ + 1], op=OP.mult)
    V.tensor_tensor(out=sm[:, DET], in0=sm[:, DET], in1=sm[:, MA], op=OP.add)
    V.tensor_tensor(out=sm[:, MB], in0=h(5), in1=sm[:, ADJ + 2], op=OP.mult)
    V.tensor_tensor(out=sm[:, DET], in0=sm[:, DET], in1=sm[:, MB], op=OP.add)
    V.reciprocal(out=sm[:, RDET], in_=sm[:, DET])
    # A = adj * (-1/det) as bf16, written straight into row-major 3x3 rows
    # [00,01,02,10,11,12,20,21,22]: upper entries by STT, mirrors by copy
    V.scalar_tensor_tensor(out=a9[:, 0:3], in0=sm[:, ADJ:ADJ + 3], scalar=-1.0,
                           in1=sm[:, RDET].unsqueeze(1).broadcast_to((P, 3, J)),
                           op0=OP.mult, op1=OP.mult)
    V.scalar_tensor_tensor(out=a9[:, 4:6], in0=sm[:, ADJ + 3:ADJ + 5], scalar=-1.0,
                           in1=sm[:, RDET].unsqueeze(1).broadcast_to((P, 2, J)),
                           op0=OP.mult, op1=OP.mult)
    V.scalar_tensor_tensor(out=a9[:, 8], in0=sm[:, ADJ + 5], scalar=-1.0,
                           in1=sm[:, RDET], op0=OP.mult, op1=OP.mult)
    V.tensor_copy(out=a9[:, 3], in_=a9[:, 1])
    V.tensor_copy(out=a9[:, 6], in_=a9[:, 2])
    V.tensor_copy(out=a9[:, 7], in_=a9[:, 5])

    # ---- bot = A @ Hth (full J: keeps the big product op 3D-mergeable) ----
    m9v = m9[:].rearrange("p (r c) a j -> p r c a j", r=3)
    a9v = a9[:].rearrange("p (r c) j -> p r c j", r=3).unsqueeze(3) \
        .broadcast_to((P, 3, 3, A, J))
    hthv = hth[:].unsqueeze(1).broadcast_to((P, 3, 3, A, J))
    V.tensor_tensor(out=m9v, in0=a9v, in1=hthv, op=OP.mult)

    # ---- bot sums, top = -(SM/TM) jac + c x bot, per J-half --------------
    for hj, hjs in ((0, slice(0, JH)), (1, slice(JH, J))):
        SMnb = ctile[:, 5, :, hjs].unsqueeze(1).broadcast_to((P, 3, A, JH))
        V.tensor_tensor(out=outt[:, hj, 3:6], in0=m9[:, 0:9:3, :, hjs],
                        in1=m9[:, 1:9:3, :, hjs], op=OP.add)
        V.tensor_tensor(out=outt[:, hj, 3:6], in0=outt[:, hj, 3:6],
                        in1=m9[:, 2:9:3, :, hjs], op=OP.add)
        V.tensor_copy(out=outt[:, hj, 6:8], in_=outt[:, hj, 3:5])
        cbb1 = cb5[:, 1:4, hjs].unsqueeze(2).broadcast_to((P, 3, A, JH))
        cbb2 = cb5[:, 2:5, hjs].unsqueeze(2).broadcast_to((P, 3, A, JH))
        V.tensor_tensor(out=scr1[:, :, :, hjs], in0=cbb1,
                        in1=outt[:, hj, 5:8], op=OP.mult)
        V.tensor_tensor(out=scr2[:, :, :, hjs], in0=cbb2,
                        in1=outt[:, hj, 4:7], op=OP.mult)
        V.tensor_tensor(out=scr1[:, :, :, hjs], in0=scr1[:, :, :, hjs],
                        in1=scr2[:, :, :, hjs], op=OP.subtract)
        V.tensor_tensor(out=scr2[:, :, :, hjs], in0=SMnb,
                        in1=jac5[:, 0:3, :, hjs], op=OP.mult)
        V.tensor_tensor(out=outt[:, hj, 0:3], in0=scr1[:, :, :, hjs],
                        in1=scr2[:, :, :, hjs], op=OP.add)
        nc.sync.dma_start(out=dram["out"][:, hj], in_=outt[:, hj, 0:6])


@functools.lru_cache(maxsize=1)
def _program():
    from contextlib import ExitStack
    import concourse.bacc as bacc
    import concourse.tile as tile
    from concourse import mybir

    f32 = mybir.dt.float32
    b16 = mybir.dt.bfloat16
    nc = bacc.Bacc("TRN2", target_bir_lowering=False, debug=False)
    JH = J // 2
    dram = {
        "rot": nc.dram_tensor("rot", [P, 3, A, J], b16, kind="ExternalInput"),
        "pos": nc.dram_tensor("pos", [P, 3, A, J], b16, kind="ExternalInput"),
        "com": nc.dram_tensor("com", [P, 3, A, J], b16, kind="ExternalInput"),
        "ctm": nc.dram_tensor("ctm", [P, 1, A, J], b16, kind="ExternalInput"),
        "ctr": nc.dram_tensor("ctr", [P, 5, A, J], b16, kind="ExternalInput"),
        "c32": nc.dram_tensor("c32", [P, NC32], f32, kind="ExternalInput"),
        "out": nc.dram_tensor("out", [P, 2, 6, A, JH], b16, kind="ExternalOutput"),
    }
    with tile.TileContext(nc) as tc:
        with ExitStack() as ctx:
            _emit(nc, tc, ctx, dram)
    nc.compile()
    return nc


def pack_inputs(com_list, link_pose_list):
    """Host-side layout packing (pure data movement + dtype cast)."""
    N = N_CORES * P * J
    pose = np.ascontiguousarray(link_pose_list, dtype=np.float32).reshape(N, 4, 4, 9)
    com = np.ascontiguousarray(com_list, dtype=np.float32).reshape(N, 3, 7)
    rot = pose[:, :3, AXIS, np.arange(7)]                # (N, 3, 7)
    pos = pose[:, :3, 3, :7]                             # (N, 3, 7)

    def to_core_layout(x):  # (N, 3, 7) -> (cores, P, 3, 7, J)
        return np.ascontiguousarray(
            x.reshape(N_CORES, P, J, 3, 7).transpose(0, 1, 3, 4, 2)).astype(BF)

    return to_core_layout(rot), to_core_layout(pos), to_core_layout(com)


def make_in_maps(rot, pos, com):
    ctm = np.ascontiguousarray(np.broadcast_to(CT[0:1], (P, 1, A, J)))
    ctr = np.ascontiguousarray(np.broadcast_to(CT[1:6], (P, 5, A, J)))
    c32 = np.broadcast_to(CONSTS32, (P, NC32)).copy()
    return [
        {"rot": rot[k], "pos": pos[k], "com": com[k], "ctm": ctm, "ctr": ctr,
         "c32": c32}
        for k in range(N_CORES)
    ]


def unpack_output(res):
    out = np.stack([res.results[k]["out"] for k in range(N_CORES)])  # (8,P,2,6,7,JH)
    out = out.astype(np.float32).transpose(0, 1, 2, 5, 3, 4)         # (8,P,2,JH,6,7)
    return np.ascontiguousarray(out.reshape(512, 256, 6, 7))


def _kernel_bm0(com, pose):
    # bm=0 path (not exercised by the shipped setup_inputs; numpy fallback)
    rot = pose[:, :, :3, 2, :N_ACT].copy()
    rot[..., 1] = pose[:, :, :3, 0, 1]
    rot[..., 5] = pose[:, :, :3, 0, 5]
    rot[..., 4] *= -1.0
    delp = pose[:, :, :3, 3, -2][..., None] - pose[:, :, :3, 3, :N_ACT]
    jt = np.cross(rot, delp, axis=2)
    return np.concatenate([jt, rot], axis=2).astype(np.float32)


def kernel(com_list, link_pose_list, bm):
    if not int(bm):
        return _kernel_bm0(np.asarray(com_list, np.float32),
                           np.asarray(link_pose_list, np.float32))

    from concourse.bass_utils import run_bass_kernel_spmd

    nc = _program()
    rot, pos, com = pack_inputs(com_list, link_pose_list)
    res = run_bass_kernel_spmd(nc, make_in_maps(rot, pos, com),
                               core_ids=list(range(N_CORES)))
    return unpack_output(res)
